# revision 1
# baseline (speedup 1.0000x reference)
"""RNN-T Joiner kernel for Trainium2 (Bass/Tile), SPMD over 8 NeuronCores.

Math: logits[b,t,u,v] = (enc@W_enc.T + b_enc + dec@W_dec.T + b_dec) @ W_out.T + b_out
    = A[b,t,v] + C[b,u,v]
where A = enc @ (W_out@W_enc).T  (no bias)
      C = dec @ (W_out@W_dec).T + (b_enc+b_dec)@W_out.T + b_out

The (B,T,U,512)@(512,500) product in the reference (73.7 GFLOP) collapses by
linearity into two small matmuls plus a broadcast add, leaving the kernel
output-bandwidth bound (288 MB of logits).

Sharding: data-parallel over batch B=16 -> 2 per core, no collectives.
All host-side work is layout only (slice / transpose / reshape).
"""

import numpy as np

B, T, U, D, V = 16, 300, 30, 512, 500
NCORES = 8
BL = B // NCORES  # batches per core
P = 128
DC = D // P  # 4 contraction chunks

T_CHUNKS = [(0, 128), (128, 128), (256, 44)]
U_GROUPS = [(u, 3) for u in range(0, U, 3)]
UG_MAX = 3

_CACHE = {}


def _build_program():
    from contextlib import ExitStack

    import concourse.bass as bass
    import concourse.tile as tile
    from concourse import bacc, mybir

    f32 = mybir.dt.float32

    nc = bacc.Bacc("TRN2", target_bir_lowering=False, debug=False)

    enc_t = nc.dram_tensor("enc_t", [D, BL * T], f32, kind="ExternalInput").ap()
    dec_t = nc.dram_tensor("dec_t", [D, BL * U], f32, kind="ExternalInput").ap()
    w_enc = nc.dram_tensor("w_enc", [D, D], f32, kind="ExternalInput").ap()
    w_dec = nc.dram_tensor("w_dec", [D, D], f32, kind="ExternalInput").ap()
    w_out_t = nc.dram_tensor("w_out_t", [D, V], f32, kind="ExternalInput").ap()
    b_enc_c = nc.dram_tensor("b_enc_c", [D, 1], f32, kind="ExternalInput").ap()
    b_dec_c = nc.dram_tensor("b_dec_c", [D, 1], f32, kind="ExternalInput").ap()
    b_out_r = nc.dram_tensor("b_out_r", [1, V], f32, kind="ExternalInput").ap()
    iota_d = nc.dram_tensor("iota_d", [BL * U, P], f32, kind="ExternalInput").ap()
    out = nc.dram_tensor("out", [BL, T, U, V], f32, kind="ExternalOutput").ap()

    with tile.TileContext(nc) as tc:
        with ExitStack() as ctx:
            persist = ctx.enter_context(tc.tile_pool(name="persist", bufs=1))

            def _tile(shape, dtype, name):
                return persist.tile(shape, dtype, name=name, tag=name)

            # ---- persistent SBUF tensors ----
            enc_sb = [_tile([P, BL * T], f32, name=f"enc_sb{i}") for i in range(DC)]
            dec_sb = [_tile([P, BL * U], f32, name=f"dec_sb{i}") for i in range(DC)]
            wenc_sb = [_tile([P, D], f32, name=f"wenc_sb{i}") for i in range(DC)]
            wdec_sb = [_tile([P, D], f32, name=f"wdec_sb{i}") for i in range(DC)]
            woutT_sb = [_tile([P, V], f32, name=f"woutT_sb{i}") for i in range(DC)]
            benc_sb = [_tile([P, 1], f32, name=f"benc_sb{i}") for i in range(DC)]
            bdec_sb = [_tile([P, 1], f32, name=f"bdec_sb{i}") for i in range(DC)]
            bout_sb = _tile([1, V], f32, name="bout_sb")
            ones_sb = _tile([1, P], f32, name="ones_sb")
            wceT_sb = [_tile([P, V], f32, name=f"wceT_sb{i}") for i in range(DC)]
            wcdT_sb = [_tile([P, V], f32, name=f"wcdT_sb{i}") for i in range(DC)]
            a_sb = [_tile([P, V], f32, name=f"a_sb{i}") for i in range(BL * len(T_CHUNKS))]
            c_sb = _tile([BL * U, V], f32, name="c_sb")
            iota_sb = _tile([BL * U, P], f32, name="iota_sb")
            bias_sb = _tile([1, V], f32, name="bias_sb")

            # ---- input DMAs ----
            for i in range(DC):
                sl = slice(i * P, (i + 1) * P)
                nc.sync.dma_start(woutT_sb[i][:], w_out_t[sl, :])
                nc.scalar.dma_start(wdec_sb[i][:], w_dec[sl, :])
            nc.sync.dma_start(iota_sb[:], iota_d[:])
            for i in range(DC):
                sl = slice(i * P, (i + 1) * P)
                nc.scalar.dma_start(dec_sb[i][:], dec_t[sl, :])
                nc.sync.dma_start(wenc_sb[i][:], w_enc[sl, :])
                nc.scalar.dma_start(enc_sb[i][:], enc_t[sl, :])
                nc.sync.dma_start(benc_sb[i][:], b_enc_c[sl, :])
                nc.sync.dma_start(bdec_sb[i][:], b_dec_c[sl, :])
            nc.sync.dma_start(bout_sb[:], b_out_r[:])
            nc.any.memset(ones_sb[:], 1.0)

            # ---- setup compute: fused weights, bias row, A, C ----
            with tc.tile_pool(name="psum_s", bufs=2, space="PSUM") as psum_s:
                # WceT[d,v] = sum_j W_enc[j,d] * W_outT[j,v]; same for WcdT
                for w_sb, wt_sb in ((wdec_sb, wcdT_sb), (wenc_sb, wceT_sb)):
                    for dc in range(DC):
                        ps = psum_s.tile([P, V], f32, tag="ps")
                        for jc in range(DC):
                            nc.tensor.matmul(
                                ps[:],
                                w_sb[jc][:, dc * P : (dc + 1) * P],
                                woutT_sb[jc][:],
                                start=(jc == 0),
                                stop=(jc == DC - 1),
                            )
                        nc.any.tensor_copy(wt_sb[dc][:], ps[:])

                # bias_row = (b_enc + b_dec) @ W_out.T + b_out
                ps_b = psum_s.tile([1, V], f32, tag="ps")
                for k, b_sb in enumerate(benc_sb + bdec_sb):
                    nc.tensor.matmul(
                        ps_b[:],
                        b_sb[:],
                        woutT_sb[k % DC][:],
                        start=(k == 0),
                        stop=(k == 2 * DC - 1),
                    )
                nc.any.tensor_add(bias_sb[:], ps_b[:], bout_sb[:])

                # A chunks: A[n,v] = sum_d encT[d,n] * WceT[d,v]
                for bl in range(BL):
                    for tci, (t0, tn) in enumerate(T_CHUNKS):
                        n0 = bl * T + t0
                        ps = psum_s.tile([P, V], f32, tag="ps")
                        for dc in range(DC):
                            nc.tensor.matmul(
                                ps[:tn, :],
                                enc_sb[dc][:, n0 : n0 + tn],
                                wceT_sb[dc][:],
                                start=(dc == 0),
                                stop=(dc == DC - 1),
                            )
                        a = a_sb[bl * len(T_CHUNKS) + tci]
                        nc.any.tensor_copy(a[:tn, :], ps[:tn, :])

                # C: C[m,v] = sum_d decT[d,m] * WcdT[d,v] + bias_row[v]
                ps_c = psum_s.tile([BL * U, V], f32, tag="ps")
                for dc in range(DC):
                    nc.tensor.matmul(
                        ps_c[:],
                        dec_sb[dc][:],
                        wcdT_sb[dc][:],
                        start=(dc == 0),
                        stop=False,
                    )
                nc.tensor.matmul(
                    ps_c[:],
                    ones_sb[:, : BL * U],
                    bias_sb[:],
                    start=False,
                    stop=True,
                )
                nc.any.tensor_copy(c_sb[:], ps_c[:])

            # ---- main loop: broadcast C rows, add A, stream out ----
            crep_pool = ctx.enter_context(
                tc.tile_pool(name="crep", bufs=2, space="PSUM")
            )
            creps_pool = ctx.enter_context(tc.tile_pool(name="crepsb", bufs=2))
            sel_pool = ctx.enter_context(tc.tile_pool(name="selp", bufs=8))
            out_pool = ctx.enter_context(tc.tile_pool(name="outp", bufs=8))

            # Adds split between DVE and Pool (ACT has no tensor_tensor).
            # Pool cannot read PSUM, so its groups get an ACT copy of the
            # broadcast tile into SBUF first.
            gidx = -1
            for bl in range(BL):
                for u0, un in U_GROUPS:
                    gidx += 1
                    on_pool = gidx % 3 == 1
                    crep = crep_pool.tile([P, UG_MAX, 512], f32, tag="crep")
                    for k in range(un):
                        r = bl * U + u0 + k
                        # one-hot row selector: sel[m, p] = (m == r)
                        sel = sel_pool.tile([BL * U, P], f32, tag="sel")
                        nc.vector.tensor_scalar(
                            sel[:],
                            iota_sb[:],
                            float(r),
                            None,
                            mybir.AluOpType.is_equal,
                        )
                        # crep[:, k, v] = sel.T @ C = C[r, v] on every partition
                        nc.tensor.matmul(
                            crep[:, k, :V],
                            sel[:],
                            c_sb[:],
                            start=True,
                            stop=True,
                        )
                    if on_pool:
                        crep_sb = creps_pool.tile([P, UG_MAX, V], f32, tag="csb")
                        nc.scalar.copy(
                            crep_sb[:, :un, :], crep[:, :un, :V]
                        )
                        src_crep = crep_sb
                    for tci, (t0, tn) in enumerate(T_CHUNKS):
                        a = a_sb[bl * len(T_CHUNKS) + tci]
                        ot = out_pool.tile([P, UG_MAX, V], f32, tag="ot")
                        for k in range(un):
                            if on_pool:
                                nc.gpsimd.tensor_add(
                                    ot[:tn, k, :], a[:tn, :], src_crep[:tn, k, :]
                                )
                            else:
                                nc.vector.tensor_add(
                                    ot[:tn, k, :], a[:tn, :], crep[:tn, k, :V]
                                )
                        dma_eng = nc.sync if (tci % 2 == 0) else nc.scalar
                        dma_eng.dma_start(
                            out[bl, t0 : t0 + tn, u0 : u0 + un, :],
                            ot[:tn, :un, :],
                        )

    nc.compile()
    return nc


def _host_prep(inputs):
    """Per-core input maps. Layout-only host work (slice/transpose/reshape)."""
    enc = np.ascontiguousarray(inputs["encoder_out"], dtype=np.float32)
    dec = np.ascontiguousarray(inputs["decoder_out"], dtype=np.float32)
    w_enc = np.ascontiguousarray(inputs["W_enc"], dtype=np.float32)
    w_dec = np.ascontiguousarray(inputs["W_dec"], dtype=np.float32)
    w_out_t = np.ascontiguousarray(inputs["W_out"].T, dtype=np.float32)
    b_enc_c = np.ascontiguousarray(inputs["b_enc"].reshape(D, 1), dtype=np.float32)
    b_dec_c = np.ascontiguousarray(inputs["b_dec"].reshape(D, 1), dtype=np.float32)
    b_out_r = np.ascontiguousarray(inputs["b_out"].reshape(1, V), dtype=np.float32)
    iota = np.broadcast_to(
        np.arange(BL * U, dtype=np.float32)[:, None], (BL * U, P)
    ).copy()

    in_maps = []
    for c in range(NCORES):
        b0 = c * BL
        enc_t = np.ascontiguousarray(enc[b0 : b0 + BL].reshape(BL * T, D).T)
        dec_t = np.ascontiguousarray(dec[b0 : b0 + BL].reshape(BL * U, D).T)
        in_maps.append(
            {
                "enc_t": enc_t,
                "dec_t": dec_t,
                "w_enc": w_enc,
                "w_dec": w_dec,
                "w_out_t": w_out_t,
                "b_enc_c": b_enc_c,
                "b_dec_c": b_dec_c,
                "b_out_r": b_out_r,
                "iota_d": iota,
            }
        )
    return in_maps


def get_program():
    if "nc" not in _CACHE:
        _CACHE["nc"] = _build_program()
    return _CACHE["nc"]


def kernel(**inputs) -> np.ndarray:
    from concourse.bass_utils import run_bass_kernel_spmd

    nc = get_program()
    in_maps = _host_prep(inputs)
    res = run_bass_kernel_spmd(nc, in_maps, list(range(NCORES)))
    return np.concatenate([r["out"] for r in res.results], axis=0)



# revision 5
# speedup vs baseline: 1.0062x; 1.0062x over previous
"""RNN-T Joiner kernel for Trainium2 (Bass/Tile), SPMD over 8 NeuronCores.

Math: logits[b,t,u,v] = (enc@W_enc.T + b_enc + dec@W_dec.T + b_dec) @ W_out.T + b_out
    = A[b,t,v] + C[b,u,v]
where A = enc @ (W_out@W_enc).T  (no bias)
      C = dec @ (W_out@W_dec).T + (b_enc+b_dec)@W_out.T + b_out

The (B,T,U,512)@(512,500) product in the reference (73.7 GFLOP) collapses by
linearity into two small matmuls plus a broadcast add, leaving the kernel
output-bandwidth bound.

The data path runs in fp16 (fp32 PSUM accumulation): 4x PE row rate, 2x DVE
rate, half the DMA bytes. End-to-end max rel err ~5e-4 against the fp32
reference (gate is 2e-2).

Per core (2 batches):
  - fuse weights on PE, compute A (600x500) and C (60x500)
  - stage C per batch into one partition row, then broadcast it to all 128
    partitions with a stride-0-source DMA (crep[p,u,v] = C[u,v] for all p)
  - per (batch, t-chunk): ONE tensor_add of shape (tn,30,500) with the A
    operand broadcast over u via a stride-0 AP, then ONE fully contiguous
    3.84MB output DMA (6 output DMAs total per core)

Sharding: data-parallel over batch B=16 -> 2 per core, no collectives.
All host-side work is layout only (slice / transpose / reshape / dtype).
"""

import numpy as np

B, T, U, D, V = 16, 300, 30, 512, 500
NCORES = 8
BL = B // NCORES  # batches per core
P = 128
DC = D // P  # 4 contraction chunks

T_CHUNKS = [(0, 128), (128, 128), (256, 44)]

_CACHE = {}


def _build_program():
    from contextlib import ExitStack

    import concourse.bass as bass
    import concourse.tile as tile
    from concourse import bacc, mybir

    f16 = mybir.dt.float16
    f32 = mybir.dt.float32

    nc = bacc.Bacc("TRN2", target_bir_lowering=False, debug=False)

    enc_t = nc.dram_tensor("enc_t", [D, BL * T], f16, kind="ExternalInput").ap()
    dec_t = nc.dram_tensor("dec_t", [D, BL * U], f16, kind="ExternalInput").ap()
    w_enc = nc.dram_tensor("w_enc", [D, D], f16, kind="ExternalInput").ap()
    w_dec = nc.dram_tensor("w_dec", [D, D], f16, kind="ExternalInput").ap()
    w_out_t = nc.dram_tensor("w_out_t", [D, V], f16, kind="ExternalInput").ap()
    b_enc_c = nc.dram_tensor("b_enc_c", [D, 1], f16, kind="ExternalInput").ap()
    b_dec_c = nc.dram_tensor("b_dec_c", [D, 1], f16, kind="ExternalInput").ap()
    b_out_r = nc.dram_tensor("b_out_r", [1, V], f16, kind="ExternalInput").ap()
    out = nc.dram_tensor("out", [BL, T, U, V], f16, kind="ExternalOutput").ap()

    with tile.TileContext(nc) as tc:
        with ExitStack() as ctx:
            persist = ctx.enter_context(tc.tile_pool(name="persist", bufs=1))

            def _tile(shape, dtype, name):
                return persist.tile(shape, dtype, name=name, tag=name)

            # ---- persistent SBUF tensors (live through the main loop) ----
            wceT_sb = [_tile([P, V], f16, name=f"wceT_sb{i}") for i in range(DC)]
            wcdT_sb = [_tile([P, V], f16, name=f"wcdT_sb{i}") for i in range(DC)]
            a_sb = [
                _tile([P, 1, V], f16, name=f"a_sb{i}") for i in range(BL * len(T_CHUNKS))
            ]
            c_sb = _tile([BL * U, V], f16, name="c_sb")
            crep = [_tile([P, U, V], f16, name=f"crep{i}") for i in range(BL)]
            bias_sb = _tile([1, V], f16, name="bias_sb")
            ones_sb = _tile([1, BL * U], f16, name="ones_sb")

            with tc.tile_pool(name="setup", bufs=1) as setup_pool:
                enc_sb = [
                    setup_pool.tile([P, BL * T], f16, name=f"enc_sb{i}", tag=f"enc{i}")
                    for i in range(DC)
                ]
                dec_sb = [
                    setup_pool.tile([P, BL * U], f16, name=f"dec_sb{i}", tag=f"dec{i}")
                    for i in range(DC)
                ]
                wenc_sb = [
                    setup_pool.tile([P, D], f16, name=f"wenc_sb{i}", tag=f"we{i}")
                    for i in range(DC)
                ]
                wdec_sb = [
                    setup_pool.tile([P, D], f16, name=f"wdec_sb{i}", tag=f"wd{i}")
                    for i in range(DC)
                ]
                woutT_sb = [
                    setup_pool.tile([P, V], f16, name=f"woutT_sb{i}", tag=f"wo{i}")
                    for i in range(DC)
                ]
                benc_sb = [
                    setup_pool.tile([P, 1], f16, name=f"benc_sb{i}", tag=f"be{i}")
                    for i in range(DC)
                ]
                bdec_sb = [
                    setup_pool.tile([P, 1], f16, name=f"bdec_sb{i}", tag=f"bd{i}")
                    for i in range(DC)
                ]
                bout_sb = setup_pool.tile([1, V], f16, name="bout_sb", tag="bo")
                c_flat = [
                    setup_pool.tile([1, U * V], f16, name=f"c_flat{i}", tag=f"cf{i}")
                    for i in range(BL)
                ]

                # ---- input DMAs (alternate the two HWDGE queues) ----
                for i in range(DC):
                    sl = slice(i * P, (i + 1) * P)
                    nc.sync.dma_start(woutT_sb[i][:], w_out_t[sl, :])
                    nc.scalar.dma_start(wenc_sb[i][:], w_enc[sl, :])
                    nc.sync.dma_start(wdec_sb[i][:], w_dec[sl, :])
                    nc.scalar.dma_start(enc_sb[i][:], enc_t[sl, :])
                    nc.sync.dma_start(dec_sb[i][:], dec_t[sl, :])
                    nc.scalar.dma_start(benc_sb[i][:], b_enc_c[sl, :])
                    nc.sync.dma_start(bdec_sb[i][:], b_dec_c[sl, :])
                nc.scalar.dma_start(bout_sb[:], b_out_r[:])
                nc.any.memset(ones_sb[:], 1.0)

                with tc.tile_pool(name="psum_s", bufs=4, space="PSUM") as psum_s:
                    # WceT[d,v] = sum_j W_enc[j,d] * W_outT[j,v]; same for WcdT
                    for w_sb, wt_sb in ((wdec_sb, wcdT_sb), (wenc_sb, wceT_sb)):
                        for dc in range(DC):
                            ps = psum_s.tile([P, V], f32, tag="ps")
                            for jc in range(DC):
                                nc.tensor.matmul(
                                    ps[:],
                                    w_sb[jc][:, dc * P : (dc + 1) * P],
                                    woutT_sb[jc][:],
                                    start=(jc == 0),
                                    stop=(jc == DC - 1),
                                )
                            nc.scalar.copy(wt_sb[dc][:], ps[:])

                    # bias_row = (b_enc + b_dec) @ W_out.T + b_out
                    ps_b = psum_s.tile([1, V], f32, tag="ps")
                    for k, b_sb in enumerate(benc_sb + bdec_sb):
                        nc.tensor.matmul(
                            ps_b[:],
                            b_sb[:],
                            woutT_sb[k % DC][:],
                            start=(k == 0),
                            stop=(k == 2 * DC - 1),
                        )
                    nc.vector.tensor_add(bias_sb[:], ps_b[:], bout_sb[:])

                    # C[m,v] = sum_d decT[d,m] * WcdT[d,v] + bias_row[v]
                    ps_c = psum_s.tile([BL * U, V], f32, tag="ps")
                    for dc in range(DC):
                        nc.tensor.matmul(
                            ps_c[:],
                            dec_sb[dc][:],
                            wcdT_sb[dc][:],
                            start=(dc == 0),
                            stop=False,
                        )
                    nc.tensor.matmul(
                        ps_c[:],
                        ones_sb[:],
                        bias_sb[:],
                        start=False,
                        stop=True,
                    )
                    nc.scalar.copy(c_sb[:], ps_c[:])

                    # Stage C rows of each batch into a single partition row,
                    # then broadcast to all 128 partitions (stride-0 source).
                    for bl in range(BL):
                        nc.sync.dma_start(
                            c_flat[bl][:], c_sb[bl * U : (bl + 1) * U, :]
                        )
                        nc.gpsimd.partition_broadcast(
                            crep[bl][:, :, :], c_flat[bl][:, :]
                        )

                    # A chunks: A[n,v] = sum_d encT[d,n] * WceT[d,v]
                    for bl in range(BL):
                        for tci, (t0, tn) in enumerate(T_CHUNKS):
                            n0 = bl * T + t0
                            ps = psum_s.tile([P, V], f32, tag="ps")
                            for dc in range(DC):
                                nc.tensor.matmul(
                                    ps[:tn, :],
                                    enc_sb[dc][:, n0 : n0 + tn],
                                    wceT_sb[dc][:],
                                    start=(dc == 0),
                                    stop=(dc == DC - 1),
                                )
                            a = a_sb[bl * len(T_CHUNKS) + tci]
                            nc.scalar.copy(a[:tn, 0, :], ps[:tn, :])

            # ---- main loop: ot[t,u,v] = A[t,v] + C[u,v], one add + one DMA
            # per (batch, t-chunk). The A operand broadcasts over u via a
            # stride-0 AP; adds split between DVE (fast, fp16 2x) and Pool.
            out_pool = ctx.enter_context(tc.tile_pool(name="outp", bufs=3))

            add_engines = [nc.vector, nc.gpsimd, nc.vector, nc.gpsimd, nc.vector,
                           nc.vector]
            gi = 0
            for bl in range(BL):
                for tci, (t0, tn) in enumerate(T_CHUNKS):
                    a = a_sb[bl * len(T_CHUNKS) + tci]
                    ot = out_pool.tile([P, U, V], f16, tag="ot")
                    add_engines[gi].tensor_add(
                        ot[:tn, :, :],
                        a[:tn, :, :].to_broadcast((tn, U, V)),
                        crep[bl][:tn, :, :],
                    )
                    dma_eng = nc.sync if (gi % 2 == 0) else nc.scalar
                    dma_eng.dma_start(
                        out[bl, t0 : t0 + tn, :, :],
                        ot[:tn, :, :],
                    )
                    gi += 1

    nc.compile()
    return nc


def _host_prep(inputs):
    """Per-core input maps. Layout-only host work (slice/transpose/dtype)."""
    enc = np.asarray(inputs["encoder_out"], dtype=np.float32)
    dec = np.asarray(inputs["decoder_out"], dtype=np.float32)
    w_enc = np.asarray(inputs["W_enc"], dtype=np.float16)
    w_dec = np.asarray(inputs["W_dec"], dtype=np.float16)
    w_out_t = np.ascontiguousarray(
        np.asarray(inputs["W_out"], dtype=np.float32).T
    ).astype(np.float16)
    b_enc_c = np.asarray(inputs["b_enc"], dtype=np.float16).reshape(D, 1)
    b_dec_c = np.asarray(inputs["b_dec"], dtype=np.float16).reshape(D, 1)
    b_out_r = np.asarray(inputs["b_out"], dtype=np.float16).reshape(1, V)

    in_maps = []
    for c in range(NCORES):
        b0 = c * BL
        enc_t = np.ascontiguousarray(enc[b0 : b0 + BL].reshape(BL * T, D).T).astype(
            np.float16
        )
        dec_t = np.ascontiguousarray(dec[b0 : b0 + BL].reshape(BL * U, D).T).astype(
            np.float16
        )
        in_maps.append(
            {
                "enc_t": enc_t,
                "dec_t": dec_t,
                "w_enc": w_enc,
                "w_dec": w_dec,
                "w_out_t": w_out_t,
                "b_enc_c": b_enc_c,
                "b_dec_c": b_dec_c,
                "b_out_r": b_out_r,
            }
        )
    return in_maps


def get_program():
    if "nc" not in _CACHE:
        _CACHE["nc"] = _build_program()
    return _CACHE["nc"]


def _get_executor():
    """Build (once) a jitted shard_map executable around the compiled Bass
    program, mirroring bass2jax.run_bass_via_pjrt's multi-core branch but
    cached across kernel() calls: no per-call retracing, donated output
    buffers created on-device, and the global output returned without a
    host-side per-core split+concat."""
    if "exec" in _CACHE:
        return _CACHE["exec"]

    import jax
    from jax.sharding import Mesh, NamedSharding, PartitionSpec
    from jax.experimental.shard_map import shard_map

    import concourse.mybir as mybir
    from concourse.bass2jax import (
        _bass_exec_p,
        install_neuronx_cc_hook,
        partition_id_tensor,
    )

    install_neuronx_cc_hook()
    nc = get_program()

    partition_name = nc.partition_id_tensor.name if nc.partition_id_tensor else None
    in_names, out_names, out_avals, zero_shapes = [], [], [], []
    for alloc in nc.m.functions[0].allocations:
        if not isinstance(alloc, mybir.MemoryLocationSet):
            continue
        name = alloc.memorylocations[0].name
        if alloc.kind == "ExternalInput":
            if name != partition_name:
                in_names.append(name)
        elif alloc.kind == "ExternalOutput":
            out_names.append(name)
            shape = tuple(alloc.tensor_shape)
            dtype = mybir.dt.np(alloc.dtype)
            out_avals.append(jax.core.ShapedArray(shape, dtype))
            zero_shapes.append((shape, dtype))
    n_params = len(in_names)
    n_outs = len(out_avals)
    all_in_names = list(in_names) + list(out_names)
    if partition_name is not None:
        all_in_names.append(partition_name)

    def _body(*args):
        operands = list(args)
        if partition_name is not None:
            operands.append(partition_id_tensor())
        outs = _bass_exec_p.bind(
            *operands,
            out_avals=tuple(out_avals),
            in_names=tuple(all_in_names),
            out_names=tuple(out_names),
            lowering_input_output_aliases=(),
            sim_require_finite=True,
            sim_require_nnan=True,
            nc=nc,
        )
        return tuple(outs)

    devices = jax.devices()[:NCORES]
    mesh = Mesh(np.asarray(devices), ("core",))
    spec = PartitionSpec("core")
    sharding = NamedSharding(mesh, spec)
    donate = tuple(range(n_params, n_params + n_outs))
    sharded = jax.jit(
        shard_map(
            _body,
            mesh=mesh,
            in_specs=(spec,) * (n_params + n_outs),
            out_specs=(spec,) * n_outs,
            check_rep=False,
        ),
        donate_argnums=donate,
        keep_unused=True,
    )

    def make_zeros():
        outs = []
        for shape, dtype in zero_shapes:
            gshape = (NCORES * shape[0],) + tuple(shape[1:])
            z = jax.jit(
                lambda s=gshape, d=dtype: jax.numpy.zeros(s, d),
                out_shardings=sharding,
            )()
            outs.append(z)
        jax.block_until_ready(outs)
        return outs

    exe = {
        "jax": jax,
        "sharded": sharded,
        "make_zeros": make_zeros,
        "in_names": in_names,
        "sharding": sharding,
    }
    _CACHE["exec"] = exe
    return exe


def kernel(**inputs) -> np.ndarray:
    exe = _get_executor()
    jax = exe["jax"]
    in_maps = _host_prep(inputs)
    concat_in = [
        np.concatenate([m[name] for m in in_maps], axis=0) for name in exe["in_names"]
    ]
    in_dev = [jax.device_put(a, exe["sharding"]) for a in concat_in]
    zeros = exe["make_zeros"]()
    outs = exe["sharded"](*in_dev, *zeros)
    jax.block_until_ready(outs)
    # shard_map concatenates per-core outputs along axis 0: (NCORES*BL, T, U, V)
    # with cores in batch order, i.e. exactly the full (B, T, U, V) output.
    res = np.asarray(outs[0])
    return res.astype(np.float32)
